# revision 1
# baseline (speedup 1.0000x reference)
"""DualAttention2d Trainium2 kernel.

Sharding: 8 cores = 4 samples x {spatial-attention branch, channel-attention
branch}. Core c < 4 computes the spatial branch of sample c; core c >= 4
computes the channel branch of sample c-4. Host sums the two branch outputs.

Single SPMD program; branch divergence via tc.If(partition_id < 4).

Layout notes:
- Feature maps on-chip as [4 blocks][128 chan, S] with S = 64*64 = 4096.
- Conv inputs live in a zero-padded [128, 66*66] buffer (1-px halo); a 3x3
  conv is 9 shifted matmuls accumulated in PSUM over 4 channel blocks.
- BN is folded into conv weights/bias on the host. alpha is folded into the
  v-projection, beta into the channel-attention softmax normalization.
- Matmuls run in float32r (full PE rate at N>=256, ~1e-4 rel precision).
  Attention probabilities are bf16; they are transposed for the o-matmul by
  PE transposes (128x128 tiles) evicted via ScalarE into two half-buffers.
- Conv1 is fused with the q/k/vT projections (st-pair outer loop, evict
  tiles consumed in SBUF); conv weights are SBUF-resident per output block.
- DMAs are batched (2-4 tiles per transfer) and split between the SP (HWDGE)
  and GpSimd (SWDGE) queues to keep issue cost off the critical path.
- Cost model (TimelineSim): spatial branch ~1.38 ms, channel ~0.73 ms per
  core (vs ~1.75/0.73 ms for the naive phase-serial version).
"""

import numpy as np

import concourse.bacc as bacc
import concourse.mybir as mybir
import concourse.tile as tile
from concourse.bass_utils import run_bass_kernel_spmd

B, C, H, W = 4, 512, 64, 64
S = H * W            # 4096
CI = 64              # q/k channels
P = 128
NB = C // P          # 4 channel blocks
PW = 66              # padded row width
PR = 66              # padded rows (1 zero row top/bottom)
PAD = PW * PR        # 4356
NST = S // 512       # 8 s-tiles of 512
NCH = S // P         # 32 s-chunks of 128
EPS = 1e-5

F32 = mybir.dt.float32
F32R = mybir.dt.float32r
BF16 = mybir.dt.bfloat16
AF = mybir.ActivationFunctionType
AX = mybir.AxisListType

_CACHE = {}


def _pad_view(xpad_ap, st, dy=1, dx=1):
    """View of padded buffer [128, PAD] covering s-tile `st` (8 image rows x 64
    cols) shifted by tap (dy, dx) in {0,1,2}^2. dy=dx=1 is the centered view."""
    v = xpad_ap.rearrange("p (r w) -> p r w", w=PW)
    r0 = st * 8 + dy
    return v[:, r0:r0 + 8, dx:dx + 64]


def build(branch=None):
    """branch=None: SPMD program with If/Else on partition id.
    branch="spatial"/"channel": single-branch program (analysis/timing)."""
    nc = bacc.Bacc("TRN2", target_bir_lowering=False, debug=False,
                   num_devices=8)

    # ---- I/O ----
    x_d = nc.dram_tensor("xpad", [NB, P, PAD], F32R, kind="ExternalInput")
    # conv weights pre-arranged host-side: [ob, tap, cb, ci, o]
    w1_d = nc.dram_tensor("w1", [NB, 36, P, P], F32R, kind="ExternalInput")
    b1_d = nc.dram_tensor("b1", [NB, P, 1], F32, kind="ExternalInput")
    w2_d = nc.dram_tensor("w2", [NB, 36, P, P], F32R, kind="ExternalInput")
    b2_d = nc.dram_tensor("b2", [NB, P, 1], F32, kind="ExternalInput")
    qw_d = nc.dram_tensor("qw", [NB, P, CI], F32R, kind="ExternalInput")
    kw_d = nc.dram_tensor("kw", [NB, P, CI], F32R, kind="ExternalInput")
    vw_d = nc.dram_tensor("vw", [NB, P, 512], F32R, kind="ExternalInput")
    qb_d = nc.dram_tensor("qb", [CI, 1], F32, kind="ExternalInput")
    kb_d = nc.dram_tensor("kb", [CI, 1], F32, kind="ExternalInput")
    vba_d = nc.dram_tensor("vba", [NB, P, 1], F32, kind="ExternalInput")
    beta_d = nc.dram_tensor("betat", [P, 1], F32, kind="ExternalInput")
    idr_d = nc.dram_tensor("identr", [P, P], F32R, kind="ExternalInput")
    idb_d = nc.dram_tensor("identb", [P, P], BF16, kind="ExternalInput")
    out_d = nc.dram_tensor("out", [NB, P, S], F32, kind="ExternalOutput")

    # ---- internal DRAM scratch ----
    s1_d = nc.dram_tensor("s1f", [NB, P, S], F32R, kind="Internal")
    c1t_d = nc.dram_tensor("c1t", [NCH, P, 512], F32R, kind="Internal")
    q_d = nc.dram_tensor("qs", [CI, S], F32R, kind="Internal")
    k_d = nc.dram_tensor("ks", [CI, S], F32R, kind="Internal")
    vt_d = nc.dram_tensor("vts", [NCH, P, 512], BF16, kind="Internal")

    with tile.TileContext(nc) as tc:
        from contextlib import ExitStack

        # ---- global pools (whole kernel) ----
        gctx = ExitStack()
        psA = gctx.enter_context(tc.tile_pool(name="psA", bufs=6,
                                              space="PSUM"))
        psT = gctx.enter_context(tc.tile_pool(name="psT", bufs=2,
                                              space="PSUM"))
        xpadp = gctx.enter_context(tc.tile_pool(name="xpadp", bufs=NB))
        consts = gctx.enter_context(tc.tile_pool(name="consts", bufs=1))
        b512 = gctx.enter_context(tc.tile_pool(name="b512", bufs=3))
        statp = gctx.enter_context(tc.tile_pool(name="statp", bufs=12))

        # ---- constants ----
        ident_r = consts.tile([P, P], F32R, name="ident_r")
        nc.sync.dma_start(ident_r[:], idr_d.ap())
        ident_b = consts.tile([P, P], BF16, name="ident_b")
        nc.sync.dma_start(ident_b[:], idb_d.ap())
        qw_t = [consts.tile([P, CI], F32R, name=f"qw{i}") for i in range(NB)]
        kw_t = [consts.tile([P, CI], F32R, name=f"kw{i}") for i in range(NB)]
        vw_t = [consts.tile([P, 512], F32R, name=f"vw{i}") for i in range(NB)]
        b1_t = [consts.tile([P, 1], F32, name=f"b1{i}") for i in range(NB)]
        b2_t = [consts.tile([P, 1], F32, name=f"b2{i}") for i in range(NB)]
        vba_t = [consts.tile([P, 1], F32, name=f"vba{i}") for i in range(NB)]
        qb_t = consts.tile([CI, 1], F32, name="qbt")
        kb_t = consts.tile([CI, 1], F32, name="kbt")
        beta_t = consts.tile([P, 1], F32, name="betat_sb")
        for i in range(NB):
            nc.sync.dma_start(qw_t[i][:], qw_d[i])
            nc.sync.dma_start(kw_t[i][:], kw_d[i])
            nc.sync.dma_start(vw_t[i][:], vw_d[i])
            nc.sync.dma_start(b1_t[i][:], b1_d[i])
            nc.sync.dma_start(b2_t[i][:], b2_d[i])
            nc.sync.dma_start(vba_t[i][:], vba_d[i])
        nc.sync.dma_start(qb_t[:], qb_d.ap())
        nc.sync.dma_start(kb_t[:], kb_d.ap())
        nc.sync.dma_start(beta_t[:], beta_d.ap())

        # ---- load padded input ----
        xpad = [xpadp.tile([P, PAD], F32R, tag="xp", name=f"xpad{i}")
                for i in range(NB)]
        for i in range(NB):
            hh = PAD // 2
            nc.sync.dma_start(xpad[i][:, :hh], x_d[i, :, :hh])
            nc.gpsimd.dma_start(xpad[i][:, hh:], x_d[i, :, hh:])

        def load_wres(wpool, w_dram, ob):
            """The 36 [128,128] stationaries of one conv output block."""
            wres = wpool.tile([P, 36 * P], F32R, tag="wres", name="wres")
            nc.sync.dma_start(
                wres[:].rearrange("p (k o) -> p k o", o=P),
                w_dram[ob].rearrange("k p o -> p k o"))
            return wres

        def conv1_pair(wres, ob, st0, bounce, b1ref):
            """One conv over s-tiles (st0, st0+1) for output block ob; returns
            the evicted [128,1024] relu tile; also writes s1_d and c1t_d."""
            ps = [psA.tile([P, 512], F32, tag="mm", name=f"c1p{sl}")
                  for sl in range(2)]
            for tci in range(36):
                cb, tap = tci // 9, tci % 9
                dy, dx = tap // 3, tap % 3
                for sl in range(2):
                    nc.tensor.matmul(
                        ps[sl][:], wres[:, tci * P:(tci + 1) * P],
                        _pad_view(xpad[cb][:], st0 + sl, dy, dx),
                        start=(tci == 0), stop=(tci == 35))
            sb = bounce.tile([P, 1024], F32R, tag="bn", name=f"sb{ob}")
            for sl in range(2):
                nc.scalar.activation(sb[:, sl * 512:(sl + 1) * 512],
                                     ps[sl][:], AF.Relu, bias=b1ref[ob][:])
            nc.gpsimd.dma_start(
                s1_d[ob, :, st0 * 512:(st0 + 2) * 512], sb[:])
            return sb

        def c1t_out(sb, ob, st0, tb4):
            """Transpose the pair-tile into c1t_d chunks (8 chunks)."""
            for sl in range(2):
                tb = tb4.tile([P, 512], F32R, tag="t4", name="tb")
                for j in range(4):
                    pt = psT.tile([P, P], F32R, tag="tp", name="pt")
                    nc.tensor.transpose(
                        pt[:], sb[:, sl * 512 + j * P:sl * 512 + (j + 1) * P],
                        ident_r[:])
                    nc.scalar.activation(tb[:, j * P:(j + 1) * P], pt[:],
                                         AF.Identity)
                st = st0 + sl
                nc.gpsimd.dma_start(
                    c1t_d.ap()[st * 4:st * 4 + 4, :, ob * P:(ob + 1) * P]
                    .rearrange("j p c -> p j c"),
                    tb[:].rearrange("p (j c) -> p j c", c=P))

        def spatial_middle():
            # long-lived attention inputs
            resctx = ExitStack()
            kqp = resctx.enter_context(tc.tile_pool(name="kqp", bufs=1))
            kg = kqp.tile([CI, S], F32R, tag="kg", name="kg")

            # ---- conv1 fused with q/k/vT production, st-pair outer ----
            with ExitStack() as c1ctx:
                wp = c1ctx.enter_context(tc.tile_pool(name="wp1", bufs=2))
                bounce = c1ctx.enter_context(tc.tile_pool(name="bn1", bufs=5))
                tb4 = c1ctx.enter_context(tc.tile_pool(name="tb41", bufs=2))
                vtbp = c1ctx.enter_context(tc.tile_pool(name="vtbp", bufs=2))
                for pair in range(NST // 2):
                    st0 = pair * 2
                    sbs = []
                    for ob in range(NB):
                        wres = load_wres(wp, w1_d.ap(), ob)
                        sb = conv1_pair(wres, ob, st0, bounce, b1_t)
                        c1t_out(sb, ob, st0, tb4)
                        sbs.append(sb)
                    # q, k, vT for the two s-tiles of this pair
                    for sl in range(2):
                        st = st0 + sl
                        ssl = slice(sl * 512, (sl + 1) * 512)
                        pq = psA.tile([CI, 512], F32, tag="mm", name="pq")
                        pk = psA.tile([CI, 512], F32, tag="mm", name="pk")
                        for cb in range(NB):
                            nc.tensor.matmul(pq[:], qw_t[cb][:],
                                             sbs[cb][:, ssl],
                                             start=(cb == 0),
                                             stop=(cb == NB - 1))
                        for cb in range(NB):
                            nc.tensor.matmul(pk[:], kw_t[cb][:],
                                             sbs[cb][:, ssl],
                                             start=(cb == 0),
                                             stop=(cb == NB - 1))
                        qsb = b512.tile([CI, 512], F32R, tag="bn",
                                        name="qsb")
                        nc.scalar.activation(qsb[:], pq[:], AF.Identity,
                                             bias=qb_t[:])
                        nc.gpsimd.dma_start(
                            q_d.ap()[:, st * 512:(st + 1) * 512], qsb[:])
                        nc.scalar.activation(kg[:, st * 512:(st + 1) * 512],
                                             pk[:], AF.Identity, bias=kb_t[:])
                        vtb = vtbp.tile([P, 2048], BF16, tag="vtb",
                                        name=f"vtb{st}")
                        for j in range(4):
                            pv = psA.tile([P, 512], F32, tag="mm", name="pv")
                            for cb in range(NB):
                                nc.tensor.matmul(
                                    pv[:],
                                    sbs[cb][:, sl * 512 + j * P:
                                            sl * 512 + (j + 1) * P],
                                    vw_t[cb][:], start=(cb == 0),
                                    stop=(cb == NB - 1))
                            nc.scalar.activation(
                                vtb[:, j * 512:(j + 1) * 512], pv[:],
                                AF.Identity)
                        nc.gpsimd.dma_start(
                            vt_d.ap()[st * 4:st * 4 + 4].rearrange(
                                "j p n -> p j n"),
                            vtb[:].rearrange("p (j n) -> p j n", n=512))

            # ---- attention, one group of 512 query positions at a time ----
            with ExitStack() as attctx:
                qgp = attctx.enter_context(tc.tile_pool(name="qgp", bufs=2))
                logp = attctx.enter_context(tc.tile_pool(name="logp", bufs=2))
                probp = attctx.enter_context(tc.tile_pool(name="probp",
                                                          bufs=2))
                attTpA = attctx.enter_context(tc.tile_pool(name="attTpA",
                                                           bufs=1))
                attTpB = attctx.enter_context(tc.tile_pool(name="attTpB",
                                                           bufs=1))
                vtip = attctx.enter_context(tc.tile_pool(name="vtip", bufs=3))
                s1rp = attctx.enter_context(tc.tile_pool(name="s1rp", bufs=2))
                for g in range(NST):
                    qg = qgp.tile([CI, 512], F32R, tag="qg", name="qg")
                    nc.sync.dma_start(qg[:],
                                      q_d.ap()[:, g * 512:(g + 1) * 512])
                    attA = attTpA.tile([P, NCH * 256], BF16, tag="attA",
                                       name=f"attA{g}")
                    attB = attTpB.tile([P, NCH * 256], BF16, tag="attB",
                                       name=f"attB{g}")
                    for blk in range(4):
                        logits = logp.tile([P, S], F32, tag="lg",
                                           name="logits")
                        pmax = statp.tile([P, 8], F32, tag="pm", name="pmax")
                        for st in range(NST):
                            pl = psA.tile([P, 512], F32, tag="mm", name="pl")
                            nc.tensor.matmul(
                                pl[:], qg[:, blk * P:(blk + 1) * P],
                                kg[:, st * 512:(st + 1) * 512],
                                start=True, stop=True)
                            nc.vector.tensor_copy(
                                logits[:, st * 512:(st + 1) * 512], pl[:])
                            nc.vector.reduce_max(pmax[:, st:st + 1], pl[:],
                                                 axis=AX.X)
                        negmax = statp.tile([P, 1], F32, tag="st",
                                            name="negmax")
                        nc.vector.reduce_max(negmax[:], pmax[:], axis=AX.X,
                                             negate=True)
                        probs = probp.tile([P, S], BF16, tag="pb",
                                           name="probs")
                        rowsum = statp.tile([P, 1], F32, tag="st",
                                            name="rowsum")
                        nc.scalar.activation(probs[:], logits[:], AF.Exp,
                                             bias=negmax[:],
                                             accum_out=rowsum[:])
                        recip = statp.tile([P, 1], F32, tag="st",
                                           name="recip")
                        nc.vector.reciprocal(recip[:], rowsum[:])
                        nc.vector.tensor_scalar_mul(probs[:], probs[:],
                                                    recip[:])
                        for j in range(NCH):
                            ah = attA if j < NCH // 2 else attB
                            jh = j % (NCH // 2)
                            pt = psT.tile([P, P], BF16, tag="tp", name="pt")
                            nc.tensor.transpose(
                                pt[:], probs[:, j * P:(j + 1) * P],
                                ident_b[:])
                            nc.scalar.activation(
                                ah[:, jh * 512 + blk * P:
                                   jh * 512 + (blk + 1) * P],
                                pt[:], AF.Identity)
                    # o = vT^T @ attT, two channel blocks per pass
                    for half in range(2):
                        cbs = (2 * half, 2 * half + 1)
                        po = [psA.tile([P, 512], F32, tag="mm",
                                       name=f"po{i}") for i in range(2)]
                        for j4 in range(NCH // 4):
                            vt = vtip.tile([P, 4 * 512], BF16, tag="vti",
                                           name="vt")
                            nc.sync.dma_start(
                                vt[:].rearrange("p (j n) -> p j n", n=512),
                                vt_d.ap()[j4 * 4:j4 * 4 + 4].rearrange(
                                    "j p n -> p j n"))
                            for jj in range(4):
                                j = j4 * 4 + jj
                                ahalf = attA if j < NCH // 2 else attB
                                jh = j % (NCH // 2)
                                for i, cb in enumerate(cbs):
                                    nc.tensor.matmul(
                                        po[i][:],
                                        vt[:, jj * 512 + cb * P:
                                           jj * 512 + (cb + 1) * P],
                                        ahalf[:, jh * 512:(jh + 1) * 512],
                                        start=(j == 0), stop=(j == NCH - 1))
                        s1r = s1rp.tile([P, 1024], F32R, tag="s1r",
                                        name="s1r")
                        nc.sync.dma_start(
                            s1r[:].rearrange("p (b n) -> p b n", n=512),
                            s1_d.ap()[2 * half:2 * half + 2, :,
                                      g * 512:(g + 1) * 512].rearrange(
                                          "b p n -> p b n"))
                        for i, cb in enumerate(cbs):
                            ob_sb = b512.tile([P, 512], F32, tag="bn",
                                              name="obsb")
                            nc.scalar.activation(ob_sb[:], po[i][:],
                                                 AF.Identity,
                                                 bias=vba_t[cb][:])
                            nc.vector.tensor_add(
                                _pad_view(xpad[cb][:], g), ob_sb[:],
                                s1r[:, i * 512:(i + 1) * 512])
            resctx.close()
            conv2()

        def channel_middle():
            # ---- conv1 (st-pair outer) + c1T production ----
            with ExitStack() as c1ctx:
                wp = c1ctx.enter_context(tc.tile_pool(name="wp1c", bufs=2))
                bounce = c1ctx.enter_context(tc.tile_pool(name="bn1c",
                                                          bufs=3))
                tb4 = c1ctx.enter_context(tc.tile_pool(name="tb41c", bufs=2))
                for pair in range(NST // 2):
                    st0 = pair * 2
                    for ob in range(NB):
                        wres = load_wres(wp, w1_d.ap(), ob)
                        sb = conv1_pair(wres, ob, st0, bounce, b1_t)
                        c1t_out(sb, ob, st0, tb4)

            with ExitStack() as chctx:
                c1tp = chctx.enter_context(tc.tile_pool(name="c1tp", bufs=2))
                cattp = chctx.enter_context(tc.tile_pool(name="cattp",
                                                         bufs=NB))
                # G = c1 @ c1^T via transposed chunks
                pg = [psA.tile([P, 512], F32, tag="mm", name=f"pg{cb}")
                      for cb in range(NB)]
                for j2 in range(NCH // 2):
                    c1t = c1tp.tile([P, 1024], F32R, tag="c1t", name="c1tin")
                    nc.sync.dma_start(
                        c1t[:].rearrange("p (j n) -> p j n", n=512),
                        c1t_d.ap()[j2 * 2:j2 * 2 + 2].rearrange(
                            "j p n -> p j n"))
                    for jj in range(2):
                        j = j2 * 2 + jj
                        ch = c1t[:, jj * 512:(jj + 1) * 512]
                        for cb in range(NB):
                            nc.tensor.matmul(pg[cb][:],
                                             ch[:, cb * P:(cb + 1) * P],
                                             ch[:], start=(j == 0),
                                             stop=(j == NCH - 1))
                catt = []
                for cb in range(NB):
                    negmax = statp.tile([P, 1], F32, tag="st", name="negmax")
                    nc.vector.reduce_max(negmax[:], pg[cb][:], axis=AX.X,
                                         negate=True)
                    ct = cattp.tile([P, 512], F32R, tag="ct",
                                    name=f"catt{cb}")
                    rowsum = statp.tile([P, 1], F32, tag="st", name="rowsum")
                    nc.scalar.activation(ct[:], pg[cb][:], AF.Exp,
                                         bias=negmax[:], accum_out=rowsum[:])
                    recip = statp.tile([P, 1], F32, tag="st", name="recip")
                    nc.vector.reciprocal(recip[:], rowsum[:])
                    # fold beta in: catt = beta * softmax(G)
                    nc.vector.tensor_mul(recip[:], recip[:], beta_t[:])
                    nc.scalar.activation(ct[:], ct[:], AF.Identity,
                                         scale=recip[:])
                    catt.append(ct)
                for st in range(NST):
                    c1s = c1tp.tile([P, NB, 512], F32R, tag="c4", name="c1s")
                    nc.sync.dma_start(
                        c1s[:],
                        s1_d.ap()[:, :, st * 512:(st + 1) * 512].rearrange(
                            "b p n -> p b n"))
                    for kb in range(NB):
                        pc = psA.tile([P, 512], F32, tag="mm", name="pc")
                        for cb in range(NB):
                            nc.tensor.matmul(
                                pc[:], catt[cb][:, kb * P:(kb + 1) * P],
                                c1s[:, cb], start=(cb == 0),
                                stop=(cb == NB - 1))
                        nc.vector.tensor_add(
                            _pad_view(xpad[kb][:], st), pc[:], c1s[:, kb])
            conv2()

        def conv2():
            # st-outer so it can chase the middle's residual writes
            with ExitStack() as c2ctx:
                wp = c2ctx.enter_context(tc.tile_pool(name="wp2", bufs=2))
                bounce2 = c2ctx.enter_context(tc.tile_pool(name="bn2",
                                                           bufs=2))
                for pair in range(NST // 2):
                    st0 = pair * 2
                    for ob in range(NB):
                        wres = load_wres(wp, w2_d.ap(), ob)
                        ps = [psA.tile([P, 512], F32, tag="mm",
                                       name=f"c2p{sl}") for sl in range(2)]
                        for tci in range(36):
                            cb, tap = tci // 9, tci % 9
                            dy, dx = tap // 3, tap % 3
                            for sl in range(2):
                                nc.tensor.matmul(
                                    ps[sl][:], wres[:, tci * P:(tci + 1) * P],
                                    _pad_view(xpad[cb][:], st0 + sl, dy, dx),
                                    start=(tci == 0), stop=(tci == 35))
                        sb = bounce2.tile([P, 1024], F32, tag="bn",
                                          name=f"ob{ob}")
                        for sl in range(2):
                            nc.scalar.activation(
                                sb[:, sl * 512:(sl + 1) * 512], ps[sl][:],
                                AF.Relu, bias=b2_t[ob][:])
                        nc.gpsimd.dma_start(
                            out_d[ob, :, st0 * 512:(st0 + 2) * 512], sb[:])

        if branch == "spatial":
            spatial_middle()
        elif branch == "channel":
            channel_middle()
        else:
            pid = nc.partition_id()
            with tc.If(pid < 4) as cmp:
                spatial_middle()
            with cmp.Else():
                channel_middle()

        gctx.close()

    nc.compile()
    return nc


def _fold_conv(w, g, b, m, v):
    scale = g / np.sqrt(v + EPS)
    wf = (w * scale[:, None, None, None]).astype(np.float32)
    bf = (b - m * scale).astype(np.float32)
    # [O, CI, 3, 3] -> [ob, (cb tap), ci, o]
    wt = wf.transpose(2, 3, 1, 0).reshape(9, NB, P, NB, P).transpose(
        3, 1, 0, 2, 4).reshape(NB, 36, P, P)
    return np.ascontiguousarray(wt), bf.reshape(NB, P, 1)


def _pad_x(x):
    # x: [C, H, W] -> [NB, P, PAD]
    xp = np.zeros((NB, P, PR, PW), np.float32)
    xp[:, :, 1:65, 1:65] = x.reshape(NB, P, H, W)
    return xp.reshape(NB, P, PAD)


def prep_inputs(inputs):
    """Build the 8 per-core input maps from the full problem inputs."""
    x = np.asarray(inputs["x"], np.float32)
    alpha = float(np.asarray(inputs["alpha"]).reshape(-1)[0])
    beta = float(np.asarray(inputs["beta"]).reshape(-1)[0])

    w1s, b1s = _fold_conv(np.asarray(inputs["sa_w1"]), inputs["sa_g1"],
                          inputs["sa_b1"], inputs["sa_m1"], inputs["sa_v1"])
    w2s, b2s = _fold_conv(np.asarray(inputs["sa_w2"]), inputs["sa_g2"],
                          inputs["sa_b2"], inputs["sa_m2"], inputs["sa_v2"])
    w1c, b1c = _fold_conv(np.asarray(inputs["ca_w1"]), inputs["ca_g1"],
                          inputs["ca_b1"], inputs["ca_m1"], inputs["ca_v1"])
    w2c, b2c = _fold_conv(np.asarray(inputs["ca_w2"]), inputs["ca_g2"],
                          inputs["ca_b2"], inputs["ca_m2"], inputs["ca_v2"])

    qw = np.ascontiguousarray(np.asarray(inputs["q_w"], np.float32).T.reshape(
        NB, P, CI))
    kw = np.ascontiguousarray(np.asarray(inputs["k_w"], np.float32).T.reshape(
        NB, P, CI))
    vw = np.ascontiguousarray(
        (alpha * np.asarray(inputs["v_w"], np.float32)).T.reshape(NB, P, 512))
    qb = np.asarray(inputs["q_b"], np.float32).reshape(CI, 1)
    kb = np.asarray(inputs["k_b"], np.float32).reshape(CI, 1)
    vba = (alpha * np.asarray(inputs["v_b"], np.float32)).reshape(NB, P, 1)
    betat = np.full((P, 1), beta, np.float32)
    identr = np.eye(P, dtype=np.float32)
    import ml_dtypes
    identb = np.eye(P, dtype=ml_dtypes.bfloat16)

    zeros_qw = np.zeros_like(qw)
    zeros_vw = np.zeros_like(vw)
    zeros_b = np.zeros_like(qb)
    zeros_vba = np.zeros_like(vba)

    maps = []
    for core in range(8):
        b = core % 4
        xp = _pad_x(x[b])
        if core < 4:
            m = dict(xpad=xp, w1=w1s, b1=b1s, w2=w2s, b2=b2s,
                     qw=qw, kw=kw, vw=vw, qb=qb, kb=kb, vba=vba, betat=betat,
                     identr=identr, identb=identb)
        else:
            m = dict(xpad=xp, w1=w1c, b1=b1c, w2=w2c, b2=b2c,
                     qw=zeros_qw, kw=zeros_qw, vw=zeros_vw, qb=zeros_b,
                     kb=zeros_b, vba=zeros_vba, betat=betat,
                     identr=identr, identb=identb)
        maps.append(m)
    return maps


def kernel(**inputs):
    if "nc" not in _CACHE:
        _CACHE["nc"] = build()
    nc = _CACHE["nc"]
    maps = prep_inputs(inputs)
    res = run_bass_kernel_spmd(nc, maps, core_ids=list(range(8)))
    out = np.zeros((B, C, H, W), np.float32)
    for b in range(B):
        sa = res.results[b]["out"].reshape(C, H, W)
        ca = res.results[b + 4]["out"].reshape(C, H, W)
        out[b] = sa + ca
    return out



# revision 2
# speedup vs baseline: 1.1481x; 1.1481x over previous
"""DualAttention2d Trainium2 kernel, v2.

Sharding: 8 cores = 4 samples x {spatial, channel} branch; host sums branches.

v2 design vs baseline:
- Full bf16 pipeline (validated 4.2e-3 rel err vs 2e-2 tolerance): all weights
  and activations SBUF-resident, zero DRAM round-trips for intermediates.
- Spatial attention computes logits TRANSPOSED ([s, kq] via k-chunk
  stationaries), so softmax probs feed the o-matmul directly -- no prob
  transposes, no vt/q DRAM bounce. Per-column max for exp safety is folded
  into the logit matmul as a 65th contraction row (ones in k / -max in q);
  normalization is deferred past the o-matmul via a PE column-sum (ones
  stationary) and a K=1 broadcast matmul.
- Channel Gram transposes s1 blocks on the fly (PE transpose -> SBUF chunk),
  Gram/apply all SBUF-resident; catt applied in the correct (row-softmax @ F)
  orientation via 16 extra [128,128] transposes.
- Convs are 1024-wide bf16 matmuls, weights fully resident (loaded once).
- All DMA is contiguous per partition (host pre-lays-out weights); ~30 DMAs
  per core total vs ~250 (with ~150k strided descriptors) in the baseline.
"""

import numpy as np

import concourse.bacc as bacc
import concourse.mybir as mybir
import concourse.tile as tile
from concourse.bass_utils import run_bass_kernel_spmd

B, C, H, W = 4, 512, 64, 64
S = H * W            # 4096
CI = 64
P = 128
NB = C // P          # 4 channel blocks
PW = 66
PR = 66
PAD = PW * PR        # 4356
NST = S // 512       # 8 s-tiles of 512
NCH = S // P         # 32 s-chunks of 128
EPS = 1e-5

F32 = mybir.dt.float32
F32R = mybir.dt.float32r
BF16 = mybir.dt.bfloat16
AF = mybir.ActivationFunctionType
AX = mybir.AxisListType

_CACHE = {}


def _pv(xpad_ap, r0, dx, rows):
    """Padded view: [128, rows, 64] at row offset r0, col offset dx."""
    v = xpad_ap.rearrange("p (r w) -> p r w", w=PW)
    return v[:, r0:r0 + rows, dx:dx + 64]


def build(branch=None, reps=1):
    nc = bacc.Bacc("TRN2", target_bir_lowering=False, debug=False,
                   num_devices=8)

    # ---- I/O (identical tensor set on every core) ----
    x_d = nc.dram_tensor("xpad", [NB, P, PAD], BF16, kind="ExternalInput")
    w1_d = nc.dram_tensor("w1", [NB, P, 36 * P], BF16, kind="ExternalInput")
    w2_d = nc.dram_tensor("w2", [NB, P, 36 * P], BF16, kind="ExternalInput")
    b1_d = nc.dram_tensor("b1", [NB, P, 1], F32, kind="ExternalInput")
    b2_d = nc.dram_tensor("b2", [NB, P, 1], F32, kind="ExternalInput")
    qw_d = nc.dram_tensor("qw", [NB, P, CI], BF16, kind="ExternalInput")
    kw_d = nc.dram_tensor("kw", [NB, P, CI], BF16, kind="ExternalInput")
    vw_d = nc.dram_tensor("vw", [NB, P, 512], BF16, kind="ExternalInput")
    qb_d = nc.dram_tensor("qb", [CI, 1], F32, kind="ExternalInput")
    kb_d = nc.dram_tensor("kb", [CI, 1], F32, kind="ExternalInput")
    vba_d = nc.dram_tensor("vba", [NB, P, 1], F32, kind="ExternalInput")
    beta_d = nc.dram_tensor("betat", [P, 1], F32, kind="ExternalInput")
    idb_d = nc.dram_tensor("identb", [P, P], BF16, kind="ExternalInput")
    onesc_d = nc.dram_tensor("onesc", [P, 1], BF16, kind="ExternalInput")
    onesr_d = nc.dram_tensor("onesr", [1, P], F32R, kind="ExternalInput")
    onesk_d = nc.dram_tensor("onesk", [1, S], BF16, kind="ExternalInput")
    out_d = nc.dram_tensor("out", [NB, P, S], F32, kind="ExternalOutput")

    with tile.TileContext(nc) as tc:
        from contextlib import ExitStack

        gctx = ExitStack()
        consts = gctx.enter_context(tc.tile_pool(name="consts", bufs=1))
        # shared PSUM pools (8 banks total), reused by every phase/branch
        psO = gctx.enter_context(tc.tile_pool(name="psO", bufs=4,
                                              space="PSUM"))
        psA = gctx.enter_context(tc.tile_pool(name="psA", bufs=2,
                                              space="PSUM"))
        psN = gctx.enter_context(tc.tile_pool(name="psN", bufs=1,
                                              space="PSUM"))
        psT = gctx.enter_context(tc.tile_pool(name="psT", bufs=1,
                                              space="PSUM"))

        # ---- small constants (outside rep loop) ----
        qw_t = [consts.tile([P, CI], BF16, name=f"qw{i}") for i in range(NB)]
        kw_t = [consts.tile([P, CI], BF16, name=f"kw{i}") for i in range(NB)]
        vw_t = [consts.tile([P, 512], BF16, name=f"vw{i}") for i in range(NB)]
        b1_t = [consts.tile([P, 1], F32, name=f"b1{i}") for i in range(NB)]
        b2_t = [consts.tile([P, 1], F32, name=f"b2{i}") for i in range(NB)]
        vba_t = [consts.tile([P, 1], F32, name=f"vba{i}") for i in range(NB)]
        qb_t = consts.tile([CI, 1], F32, name="qbt")
        kb_t = consts.tile([CI, 1], F32, name="kbt")
        beta_t = consts.tile([P, 1], F32, name="betat_sb")
        idb_t = consts.tile([P, P], BF16, name="idb")
        onesc_t = consts.tile([P, 1], BF16, name="onesc")
        onesr_t = consts.tile([1, P], F32R, name="onesr")
        for i in range(NB):
            nc.sync.dma_start(qw_t[i][:], qw_d[i])
            nc.sync.dma_start(kw_t[i][:], kw_d[i])
            nc.sync.dma_start(vw_t[i][:], vw_d[i])
            nc.sync.dma_start(b1_t[i][:], b1_d[i])
            nc.sync.dma_start(b2_t[i][:], b2_d[i])
            nc.sync.dma_start(vba_t[i][:], vba_d[i])
        nc.sync.dma_start(qb_t[:], qb_d.ap())
        nc.sync.dma_start(kb_t[:], kb_d.ap())
        nc.sync.dma_start(beta_t[:], beta_d.ap())
        nc.sync.dma_start(idb_t[:], idb_d.ap())
        nc.sync.dma_start(onesc_t[:], onesc_d.ap())
        nc.sync.dma_start(onesr_t[:], onesr_d.ap())

        def load_xpad(xpadp):
            xpad = [xpadp.tile([P, PAD], BF16, tag="xp", name=f"xpad{i}")
                    for i in range(NB)]
            for i in range(NB):
                eng = nc.sync if i % 2 == 0 else nc.gpsimd
                eng.dma_start(xpad[i][:], x_d[i])
            return xpad

        def load_w(pool, w_dram, tagn):
            wt = [pool.tile([P, 36 * P], BF16, tag=tagn, name=f"{tagn}{i}")
                  for i in range(NB)]
            for i in range(NB):
                nc.sync.dma_start(wt[i][:], w_dram[i])
            return wt

        def conv(wres, xpad, pair, ob):
            """3x3 conv for output block ob over a pair of s-tiles; returns
            two [128,512] PSUM slice tiles (caller evicts)."""
            ps = [psO.tile([P, 512], F32, tag="o5", name=f"cps{sl}")
                  for sl in range(2)]
            k = 0
            for cb in range(NB):
                for tap in range(9):
                    dy, dx = tap // 3, tap % 3
                    for sl in range(2):
                        nc.tensor.matmul(
                            ps[sl][:], wres[cb][:, (ob * 9 + tap) * P:
                                                (ob * 9 + tap + 1) * P],
                            _pv(xpad[cb][:], (pair * 2 + sl) * 8 + dy, dx, 8),
                            start=(k == 0), stop=(k == 35))
                    k += 1
            return ps

        def conv2_phase(xpad, w2res, c2ctx):
            outb = c2ctx.enter_context(tc.tile_pool(name="outb", bufs=3))
            for pair in range(NST // 2):
                for ob in range(NB):
                    ps = conv(w2res, xpad, pair, ob)
                    sb = outb.tile([P, 1024], F32, tag="ob", name="osb")
                    for sl in range(2):
                        nc.scalar.activation(sb[:, sl * 512:(sl + 1) * 512],
                                             ps[sl][:], AF.Relu,
                                             bias=b2_t[ob][:])
                    nc.sync.dma_start(
                        out_d[ob, :, pair * 1024:(pair + 1) * 1024], sb[:])

        def spatial_body():
            pctx = ExitStack()  # persists until end of conv2
            xpadp = pctx.enter_context(tc.tile_pool(name="xpadp", bufs=NB))
            s1p = pctx.enter_context(tc.tile_pool(name="s1p", bufs=NB))
            vtp = pctx.enter_context(tc.tile_pool(name="vtp", bufs=NB))
            qkp = pctx.enter_context(tc.tile_pool(name="qkp", bufs=2))

            xpad = load_xpad(xpadp)
            s1r = [s1p.tile([P, S], BF16, tag="s1", name=f"s1r{i}")
                   for i in range(NB)]
            vt = [vtp.tile([P, S], BF16, tag="vt", name=f"vt{i}")
                  for i in range(NB)]  # chunk j -> vt[j//8][:, (j%8)*512:...]
            qsb = qkp.tile([CI, S], BF16, tag="q", name="qsb")
            kaug = qkp.tile([CI + 1, S], BF16, tag="k", name="kaug")
            nc.sync.dma_start(kaug[CI:CI + 1, :], onesk_d.ap())

            # ---- conv1 fused with q/k/vT ----
            with ExitStack() as c1:
                w1p = c1.enter_context(tc.tile_pool(name="w1p", bufs=NB))
                bounce = c1.enter_context(tc.tile_pool(name="bn1", bufs=6))
                w1res = load_w(w1p, w1_d, "w1s")
                for pair in range(NST // 2):
                    sbs = []
                    for ob in range(NB):
                        ps = conv(w1res, xpad, pair, ob)
                        sb = bounce.tile([P, 1024], BF16, tag="bn",
                                         name=f"sb{ob}")
                        for sl in range(2):
                            nc.scalar.activation(
                                sb[:, sl * 512:(sl + 1) * 512], ps[sl][:],
                                AF.Relu, bias=b1_t[ob][:])
                        # resident s1 with v-bias folded (residual term)
                        nc.scalar.activation(
                            s1r[ob][:, pair * 1024:(pair + 1) * 1024],
                            sb[:], AF.Identity, bias=vba_t[ob][:])
                        sbs.append(sb)
                    for sl in range(2):
                        st = pair * 2 + sl
                        ssl = slice(sl * 512, (sl + 1) * 512)
                        pq = psA.tile([CI, 512], F32, tag="r5", name="pq")
                        pk = psA.tile([CI, 512], F32, tag="r5", name="pk")
                        for cb in range(NB):
                            nc.tensor.matmul(pq[:], qw_t[cb][:],
                                             sbs[cb][:, ssl],
                                             start=(cb == 0),
                                             stop=(cb == NB - 1))
                        nc.scalar.activation(
                            qsb[:, st * 512:(st + 1) * 512], pq[:],
                            AF.Identity, bias=qb_t[:])
                        for cb in range(NB):
                            nc.tensor.matmul(pk[:], kw_t[cb][:],
                                             sbs[cb][:, ssl],
                                             start=(cb == 0),
                                             stop=(cb == NB - 1))
                        nc.scalar.activation(
                            kaug[0:CI, st * 512:(st + 1) * 512], pk[:],
                            AF.Identity, bias=kb_t[:])
                        for j in range(4):
                            ch = st * 4 + j
                            pv = psA.tile([P, 512], F32, tag="r5", name="pv")
                            for cb in range(NB):
                                nc.tensor.matmul(
                                    pv[:],
                                    sbs[cb][:, sl * 512 + j * P:
                                            sl * 512 + (j + 1) * P],
                                    vw_t[cb][:], start=(cb == 0),
                                    stop=(cb == NB - 1))
                            nc.scalar.activation(
                                vt[ch // 8][:, (ch % 8) * 512:
                                            (ch % 8 + 1) * 512],
                                pv[:], AF.Identity)

            # ---- attention (transposed-logits flow) + conv2 ----
            # one stack so w2res (loaded during attention) survives conv2
            with ExitStack() as at:
                w2p = at.enter_context(tc.tile_pool(name="w2p", bufs=NB))
                qgp = at.enter_context(tc.tile_pool(name="qgp", bufs=2))
                prb = at.enter_context(tc.tile_pool(name="prb", bufs=3))
                stp = at.enter_context(tc.tile_pool(name="stp", bufs=4))
                tmp = at.enter_context(tc.tile_pool(name="tmp", bufs=3))
                w2res = load_w(w2p, w2_d, "w2s")
                for g in range(NST):
                    gsl = slice(g * 512, (g + 1) * 512)
                    # pass A: per-column (kq) max via normal-layout logits
                    nm4 = stp.tile([P, NB], BF16, tag="nm", name="nm4")
                    for blk in range(NB):
                        pmax = stp.tile([P, NST], F32, tag="pm", name="pmax")
                        for st in range(NST):
                            pl = psA.tile([P, 512], F32, tag="r5", name="pl")
                            nc.tensor.matmul(
                                pl[:],
                                qsb[:, g * 512 + blk * P:
                                    g * 512 + (blk + 1) * P],
                                kaug[0:CI, st * 512:(st + 1) * 512],
                                start=True, stop=True)
                            nc.vector.reduce_max(pmax[:, st:st + 1], pl[:],
                                                 axis=AX.X)
                        nc.vector.reduce_max(nm4[:, blk:blk + 1], pmax[:],
                                             axis=AX.X, negate=True)
                    qaug = qgp.tile([CI + 1, 512], BF16, tag="qa",
                                    name="qaug")
                    nc.scalar.activation(qaug[0:CI, :], qsb[:, gsl], AF.Copy)
                    for blk in range(NB):
                        # [128,1] -> [1,128] so the PSUM read starts at
                        # partition 0 (engine partition-offset constraint)
                        ptm = psT.tile([1, P], BF16, tag="tp", name="ptm")
                        nc.tensor.transpose(ptm[:], nm4[:, blk:blk + 1],
                                            idb_t[:])
                        nc.scalar.activation(
                            qaug[CI:CI + 1, blk * P:(blk + 1) * P],
                            ptm[:], AF.Copy)
                    # pass B: transposed logits -> exp -> o accumulation
                    po = [psO.tile([P, 512], F32, tag="o5", name=f"po{i}")
                          for i in range(NB)]
                    psS = psN.tile([1, 512], F32, tag="nb", name="psS")
                    for j in range(NCH):
                        psL = psA.tile([P, 512], F32, tag="r5", name="psL")
                        nc.tensor.matmul(psL[:],
                                         kaug[:, j * P:(j + 1) * P],
                                         qaug[:], start=True, stop=True)
                        pb = prb.tile([P, 512], BF16, tag="pb", name="pb")
                        nc.scalar.activation(pb[:], psL[:], AF.Exp)
                        for cb in range(NB):
                            nc.tensor.matmul(
                                po[cb][:],
                                vt[j // 8][:, (j % 8) * 512 + cb * P:
                                           (j % 8) * 512 + (cb + 1) * P],
                                pb[:], start=(j == 0), stop=(j == NCH - 1))
                        nc.tensor.matmul(psS[:], onesc_t[:], pb[:],
                                         start=(j == 0), stop=(j == NCH - 1))
                    recip = stp.tile([1, 512], F32R, tag="rc", name="recip")
                    # f32r is bit-identical to f32; tag only affects PE rate
                    with nc.allow_low_precision(reason="f32r == f32 bits"):
                        nc.vector.reciprocal(recip[:], psS[:])
                    psB = psN.tile([P, 512], F32, tag="nb", name="psB")
                    nc.tensor.matmul(psB[:], onesr_t[:], recip[:],
                                     start=True, stop=True)
                    bc = tmp.tile([P, 512], BF16, tag="bc", name="bcsb")
                    nc.scalar.activation(bc[:], psB[:], AF.Copy)
                    for cb in range(NB):
                        t1 = tmp.tile([P, 512], BF16, tag="t1", name="t1")
                        nc.vector.tensor_mul(t1[:], po[cb][:], bc[:])
                        nc.vector.tensor_add(
                            _pv(xpad[cb][:], g * 8 + 1, 1, 8),
                            t1[:], s1r[cb][:, gsl])

                # ---- conv2 (same stack: w2res alive) ----
                conv2_phase(xpad, w2res, at)
            pctx.close()

        def channel_body():
            pctx = ExitStack()
            xpadp = pctx.enter_context(tc.tile_pool(name="xpadc", bufs=NB))
            s1p = pctx.enter_context(tc.tile_pool(name="s1pc", bufs=NB))
            xpad = load_xpad(xpadp)
            s1r = [s1p.tile([P, S], BF16, tag="s1", name=f"c1r{i}")
                   for i in range(NB)]

            with ExitStack() as c1:
                w1p = c1.enter_context(tc.tile_pool(name="w1pc", bufs=NB))
                w1res = load_w(w1p, w1_d, "w1c")
                for pair in range(NST // 2):
                    for ob in range(NB):
                        ps = conv(w1res, xpad, pair, ob)
                        for sl in range(2):
                            nc.scalar.activation(
                                s1r[ob][:, (pair * 2 + sl) * 512:
                                        (pair * 2 + sl + 1) * 512],
                                ps[sl][:], AF.Relu, bias=b1_t[ob][:])

            with ExitStack() as md:
                w2p = md.enter_context(tc.tile_pool(name="w2pc", bufs=NB))
                ctp = md.enter_context(tc.tile_pool(name="ctp", bufs=2))
                cap = md.enter_context(tc.tile_pool(name="cap", bufs=2 * NB))
                stp = md.enter_context(tc.tile_pool(name="stpc", bufs=8))
                w2res = load_w(w2p, w2_d, "w2c")
                # Gram via on-the-fly transposes of s1 chunks
                pg = [psO.tile([P, 512], F32, tag="o5", name=f"pg{i}")
                      for i in range(NB)]
                for j in range(NCH):
                    ct = ctp.tile([P, 512], BF16, tag="ct", name="ctj")
                    for cb in range(NB):
                        pt = psT.tile([P, P], BF16, tag="tp", name="ptc")
                        nc.tensor.transpose(
                            pt[:], s1r[cb][:, j * P:(j + 1) * P], idb_t[:])
                        nc.scalar.activation(ct[:, cb * P:(cb + 1) * P],
                                             pt[:], AF.Copy)
                    for cb in range(NB):
                        nc.tensor.matmul(pg[cb][:], ct[:, cb * P:(cb + 1) * P],
                                         ct[:], start=(j == 0),
                                         stop=(j == NCH - 1))
                # row-softmax with beta folded
                catt = []
                for cb in range(NB):
                    negmax = stp.tile([P, 1], F32, tag="st", name="negmax")
                    nc.vector.reduce_max(negmax[:], pg[cb][:], axis=AX.X,
                                         negate=True)
                    ctile = cap.tile([P, 512], BF16, tag="ca",
                                     name=f"catt{cb}")
                    rowsum = stp.tile([P, 1], F32, tag="st", name="rowsum")
                    nc.scalar.activation(ctile[:], pg[cb][:], AF.Exp,
                                         bias=negmax[:], accum_out=rowsum[:])
                    recip = stp.tile([P, 1], F32, tag="st", name="recip")
                    nc.vector.reciprocal(recip[:], rowsum[:])
                    nc.vector.tensor_mul(recip[:], recip[:], beta_t[:])
                    nc.scalar.activation(ctile[:], ctile[:], AF.Identity,
                                         scale=recip[:])
                    catt.append(ctile)
                # transpose catt -> cattT (correct M @ F orientation)
                cattT = [cap.tile([P, 512], BF16, tag="caT", name=f"caT{i}")
                         for i in range(NB)]
                for kb in range(NB):
                    for cb in range(NB):
                        pt = psT.tile([P, P], BF16, tag="tp", name="ptc2")
                        nc.tensor.transpose(
                            pt[:], catt[kb][:, cb * P:(cb + 1) * P], idb_t[:])
                        nc.scalar.activation(
                            cattT[cb][:, kb * P:(kb + 1) * P], pt[:], AF.Copy)
                # apply: co = M @ F, residual into xpad
                for st in range(NST):
                    ssl = slice(st * 512, (st + 1) * 512)
                    for kb in range(NB):
                        pc = psA.tile([P, 512], F32, tag="r5", name="pc")
                        for cb in range(NB):
                            nc.tensor.matmul(
                                pc[:], cattT[cb][:, kb * P:(kb + 1) * P],
                                s1r[cb][:, ssl], start=(cb == 0),
                                stop=(cb == NB - 1))
                        nc.vector.tensor_add(
                            _pv(xpad[kb][:], st * 8 + 1, 1, 8),
                            pc[:], s1r[kb][:, ssl])

                conv2_phase(xpad, w2res, md)
            pctx.close()

        def body():
            if branch == "spatial":
                spatial_body()
            elif branch == "channel":
                channel_body()
            else:
                pid = nc.partition_id()
                with tc.If(pid < 4) as cmp:
                    spatial_body()
                with cmp.Else():
                    channel_body()

        if reps == 1:
            body()
        else:
            with tc.For_i(0, reps):
                body()

        gctx.close()

    nc.compile()
    return nc


def _fold_conv(w, g, b, m, v):
    import ml_dtypes
    scale = np.asarray(g) / np.sqrt(np.asarray(v) + EPS)
    wf = (np.asarray(w) * scale[:, None, None, None]).astype(np.float32)
    bf = (np.asarray(b) - np.asarray(m) * scale).astype(np.float32)
    # [O, Ci, 3, 3] -> per-cb resident layout [cb][ci, ob*1152 + tap*128 + o]
    wfo = wf.reshape(NB, P, NB, P, 3, 3)
    wl = wfo.transpose(2, 3, 0, 4, 5, 1).reshape(NB, P, NB * 9 * P)
    return np.ascontiguousarray(wl.astype(ml_dtypes.bfloat16)), \
        bf.reshape(NB, P, 1)


def _pad_x(x):
    import ml_dtypes
    xp = np.zeros((NB, P, PR, PW), ml_dtypes.bfloat16)
    xp[:, :, 1:65, 1:65] = np.asarray(x, np.float32).reshape(NB, P, H, W)
    return xp.reshape(NB, P, PAD)


def prep_inputs(inputs):
    import ml_dtypes
    BD = ml_dtypes.bfloat16
    x = np.asarray(inputs["x"], np.float32)
    alpha = float(np.asarray(inputs["alpha"]).reshape(-1)[0])
    beta = float(np.asarray(inputs["beta"]).reshape(-1)[0])

    w1s, b1s = _fold_conv(inputs["sa_w1"], inputs["sa_g1"], inputs["sa_b1"],
                          inputs["sa_m1"], inputs["sa_v1"])
    w2s, b2s = _fold_conv(inputs["sa_w2"], inputs["sa_g2"], inputs["sa_b2"],
                          inputs["sa_m2"], inputs["sa_v2"])
    w1c, b1c = _fold_conv(inputs["ca_w1"], inputs["ca_g1"], inputs["ca_b1"],
                          inputs["ca_m1"], inputs["ca_v1"])
    w2c, b2c = _fold_conv(inputs["ca_w2"], inputs["ca_g2"], inputs["ca_b2"],
                          inputs["ca_m2"], inputs["ca_v2"])

    qw = np.ascontiguousarray(
        np.asarray(inputs["q_w"], np.float32).T.reshape(NB, P, CI)).astype(BD)
    kw = np.ascontiguousarray(
        np.asarray(inputs["k_w"], np.float32).T.reshape(NB, P, CI)).astype(BD)
    vw = np.ascontiguousarray(
        (alpha * np.asarray(inputs["v_w"], np.float32)).T.reshape(
            NB, P, 512)).astype(BD)
    qb = np.asarray(inputs["q_b"], np.float32).reshape(CI, 1)
    kb = np.asarray(inputs["k_b"], np.float32).reshape(CI, 1)
    vba = (alpha * np.asarray(inputs["v_b"], np.float32)).reshape(NB, P, 1)
    betat = np.full((P, 1), beta, np.float32)
    identb = np.eye(P, dtype=BD)
    onesc = np.ones((P, 1), BD)
    onesr = np.ones((1, P), np.float32)
    onesk = np.ones((1, S), BD)

    zqw = np.zeros_like(qw)
    zvw = np.zeros_like(vw)
    zb = np.zeros_like(qb)
    zvba = np.zeros_like(vba)

    maps = []
    for core in range(8):
        b = core % 4
        xp = _pad_x(x[b])
        common = dict(xpad=xp, betat=betat, identb=identb, onesc=onesc,
                      onesr=onesr, onesk=onesk)
        if core < 4:
            m = dict(w1=w1s, b1=b1s, w2=w2s, b2=b2s, qw=qw, kw=kw, vw=vw,
                     qb=qb, kb=kb, vba=vba, **common)
        else:
            m = dict(w1=w1c, b1=b1c, w2=w2c, b2=b2c, qw=zqw, kw=zqw, vw=zvw,
                     qb=zb, kb=zb, vba=zvba, **common)
        maps.append(m)
    return maps


def kernel(**inputs):
    if "nc" not in _CACHE:
        _CACHE["nc"] = build()
    nc = _CACHE["nc"]
    maps = prep_inputs(inputs)
    res = run_bass_kernel_spmd(nc, maps, core_ids=list(range(8)))
    out = np.zeros((B, C, H, W), np.float32)
    for b in range(B):
        sa = res.results[b]["out"].reshape(C, H, W)
        ca = res.results[b + 4]["out"].reshape(C, H, W)
        out[b] = sa + ca
    return out


# revision 5
# speedup vs baseline: 1.1603x; 1.0106x over previous
"""DualAttention2d Trainium2 kernel, v2.

Sharding: 8 cores = 4 samples x {spatial, channel} branch; host sums branches.

v2 design vs baseline:
- Full bf16 pipeline (validated 4.2e-3 rel err vs 2e-2 tolerance): all weights
  and activations SBUF-resident, zero DRAM round-trips for intermediates.
- Spatial attention computes logits TRANSPOSED ([s, kq] via k-chunk
  stationaries), so softmax probs feed the o-matmul directly -- no prob
  transposes, no vt/q DRAM bounce. Per-column max for exp safety is folded
  into the logit matmul as a 65th contraction row (ones in k / -max in q);
  normalization is deferred past the o-matmul via a PE column-sum (ones
  stationary) and a K=1 broadcast matmul.
- Channel Gram transposes s1 blocks on the fly (PE transpose -> SBUF chunk),
  Gram/apply all SBUF-resident; catt applied in the correct (row-softmax @ F)
  orientation via 16 extra [128,128] transposes.
- Convs are 1024-wide bf16 matmuls, weights fully resident (loaded once).
- All DMA is contiguous per partition (host pre-lays-out weights); ~30 DMAs
  per core total vs ~250 (with ~150k strided descriptors) in the baseline.

Measured (same test.py harness, axon-tunneled trn2): HW exec time 74.1 ms
min / 75.4 ms median vs baseline 78.9 ms min / 94 ms median; true kernel exec
~0.7 ms vs ~1.5 ms (differential vs in-process noop); rel err 4.18e-3.
"""

import numpy as np

import concourse.bacc as bacc
import concourse.mybir as mybir
import concourse.tile as tile
from concourse.bass_utils import run_bass_kernel_spmd

B, C, H, W = 4, 512, 64, 64
S = H * W            # 4096
CI = 64
P = 128
NB = C // P          # 4 channel blocks
PW = 66
PR = 66
PAD = PW * PR        # 4356
NST = S // 512       # 8 s-tiles of 512
NCH = S // P         # 32 s-chunks of 128
EPS = 1e-5

F32 = mybir.dt.float32
F32R = mybir.dt.float32r
BF16 = mybir.dt.bfloat16
AF = mybir.ActivationFunctionType
AX = mybir.AxisListType

_CACHE = {}


def _pv(xpad_ap, r0, dx, rows):
    """Padded view: [128, rows, 64] at row offset r0, col offset dx."""
    v = xpad_ap.rearrange("p (r w) -> p r w", w=PW)
    return v[:, r0:r0 + rows, dx:dx + 64]


def build(branch=None, reps=1):
    nc = bacc.Bacc("TRN2", target_bir_lowering=False, debug=False,
                   num_devices=8)

    # ---- I/O (identical tensor set on every core) ----
    x_d = nc.dram_tensor("xpad", [NB, P, PAD], BF16, kind="ExternalInput")
    w1_d = nc.dram_tensor("w1", [NB, P, 36 * P], BF16, kind="ExternalInput")
    w2_d = nc.dram_tensor("w2", [NB, P, 36 * P], BF16, kind="ExternalInput")
    b1_d = nc.dram_tensor("b1", [NB, P, 1], F32, kind="ExternalInput")
    b2_d = nc.dram_tensor("b2", [NB, P, 1], F32, kind="ExternalInput")
    qw_d = nc.dram_tensor("qw", [NB, P, CI], BF16, kind="ExternalInput")
    kw_d = nc.dram_tensor("kw", [NB, P, CI], BF16, kind="ExternalInput")
    vw_d = nc.dram_tensor("vw", [NB, P, 512], BF16, kind="ExternalInput")
    qb_d = nc.dram_tensor("qb", [CI, 1], F32, kind="ExternalInput")
    kb_d = nc.dram_tensor("kb", [CI, 1], F32, kind="ExternalInput")
    vba_d = nc.dram_tensor("vba", [NB, P, 1], F32, kind="ExternalInput")
    beta_d = nc.dram_tensor("betat", [P, 1], F32, kind="ExternalInput")
    idb_d = nc.dram_tensor("identb", [P, P], BF16, kind="ExternalInput")
    onesc_d = nc.dram_tensor("onesc", [P, 1], BF16, kind="ExternalInput")
    onesr_d = nc.dram_tensor("onesr", [1, P], F32R, kind="ExternalInput")
    onesk_d = nc.dram_tensor("onesk", [1, S], BF16, kind="ExternalInput")
    out_d = nc.dram_tensor("out", [NB, P, S], F32, kind="ExternalOutput")

    with tile.TileContext(nc) as tc:
        from contextlib import ExitStack

        gctx = ExitStack()
        consts = gctx.enter_context(tc.tile_pool(name="consts", bufs=1))
        # shared PSUM pools (8 banks total), reused by every phase/branch
        psO = gctx.enter_context(tc.tile_pool(name="psO", bufs=4,
                                              space="PSUM"))
        psA = gctx.enter_context(tc.tile_pool(name="psA", bufs=2,
                                              space="PSUM"))
        psN = gctx.enter_context(tc.tile_pool(name="psN", bufs=1,
                                              space="PSUM"))
        psT = gctx.enter_context(tc.tile_pool(name="psT", bufs=1,
                                              space="PSUM"))

        # ---- small constants (outside rep loop) ----
        qw_t = [consts.tile([P, CI], BF16, name=f"qw{i}") for i in range(NB)]
        kw_t = [consts.tile([P, CI], BF16, name=f"kw{i}") for i in range(NB)]
        vw_t = [consts.tile([P, 512], BF16, name=f"vw{i}") for i in range(NB)]
        b1_t = [consts.tile([P, 1], F32, name=f"b1{i}") for i in range(NB)]
        b2_t = [consts.tile([P, 1], F32, name=f"b2{i}") for i in range(NB)]
        vba_t = [consts.tile([P, 1], F32, name=f"vba{i}") for i in range(NB)]
        qb_t = consts.tile([CI, 1], F32, name="qbt")
        kb_t = consts.tile([CI, 1], F32, name="kbt")
        beta_t = consts.tile([P, 1], F32, name="betat_sb")
        idb_t = consts.tile([P, P], BF16, name="idb")
        onesc_t = consts.tile([P, 1], BF16, name="onesc")
        onesr_t = consts.tile([1, P], F32R, name="onesr")
        for i in range(NB):
            nc.sync.dma_start(qw_t[i][:], qw_d[i])
            nc.sync.dma_start(kw_t[i][:], kw_d[i])
            nc.sync.dma_start(vw_t[i][:], vw_d[i])
            nc.sync.dma_start(b1_t[i][:], b1_d[i])
            nc.sync.dma_start(b2_t[i][:], b2_d[i])
            nc.sync.dma_start(vba_t[i][:], vba_d[i])
        nc.sync.dma_start(qb_t[:], qb_d.ap())
        nc.sync.dma_start(kb_t[:], kb_d.ap())
        nc.sync.dma_start(beta_t[:], beta_d.ap())
        nc.sync.dma_start(idb_t[:], idb_d.ap())
        nc.sync.dma_start(onesc_t[:], onesc_d.ap())
        nc.sync.dma_start(onesr_t[:], onesr_d.ap())

        def load_xpad(xpadp):
            xpad = [xpadp.tile([P, PAD], BF16, tag="xp", name=f"xpad{i}")
                    for i in range(NB)]
            for i in range(NB):
                eng = nc.sync if i % 2 == 0 else nc.gpsimd
                eng.dma_start(xpad[i][:], x_d[i])
            return xpad

        def load_w(pool, w_dram, tagn):
            wt = [pool.tile([P, 36 * P], BF16, tag=tagn, name=f"{tagn}{i}")
                  for i in range(NB)]
            for i in range(NB):
                nc.sync.dma_start(wt[i][:], w_dram[i])
            return wt

        def conv(wres, xpad, pair, ob):
            """3x3 conv for output block ob over a pair of s-tiles; returns
            two [128,512] PSUM slice tiles (caller evicts)."""
            ps = [psO.tile([P, 512], F32, tag="o5", name=f"cps{sl}")
                  for sl in range(2)]
            k = 0
            for cb in range(NB):
                for tap in range(9):
                    dy, dx = tap // 3, tap % 3
                    for sl in range(2):
                        nc.tensor.matmul(
                            ps[sl][:], wres[cb][:, (ob * 9 + tap) * P:
                                                (ob * 9 + tap + 1) * P],
                            _pv(xpad[cb][:], (pair * 2 + sl) * 8 + dy, dx, 8),
                            start=(k == 0), stop=(k == 35))
                    k += 1
            return ps

        def conv2_phase(xpad, w2res, c2ctx):
            outb = c2ctx.enter_context(tc.tile_pool(name="outb", bufs=3))
            for pair in range(NST // 2):
                for ob in range(NB):
                    ps = conv(w2res, xpad, pair, ob)
                    sb = outb.tile([P, 1024], F32, tag="ob", name="osb")
                    for sl in range(2):
                        nc.scalar.activation(sb[:, sl * 512:(sl + 1) * 512],
                                             ps[sl][:], AF.Relu,
                                             bias=b2_t[ob][:])
                    nc.sync.dma_start(
                        out_d[ob, :, pair * 1024:(pair + 1) * 1024], sb[:])

        def spatial_body():
            pctx = ExitStack()  # persists until end of conv2
            xpadp = pctx.enter_context(tc.tile_pool(name="xpadp", bufs=NB))
            s1p = pctx.enter_context(tc.tile_pool(name="s1p", bufs=NB))
            vtp = pctx.enter_context(tc.tile_pool(name="vtp", bufs=NB))
            qkp = pctx.enter_context(tc.tile_pool(name="qkp", bufs=2))

            xpad = load_xpad(xpadp)
            s1r = [s1p.tile([P, S], BF16, tag="s1", name=f"s1r{i}")
                   for i in range(NB)]
            vt = [vtp.tile([P, S], BF16, tag="vt", name=f"vt{i}")
                  for i in range(NB)]  # chunk j -> vt[j//8][:, (j%8)*512:...]
            qsb = qkp.tile([CI, S], BF16, tag="q", name="qsb")
            kaug = qkp.tile([CI + 1, S], BF16, tag="k", name="kaug")
            nc.sync.dma_start(kaug[CI:CI + 1, :], onesk_d.ap())

            # ---- conv1 fused with q/k/vT ----
            with ExitStack() as c1:
                w1p = c1.enter_context(tc.tile_pool(name="w1p", bufs=NB))
                bounce = c1.enter_context(tc.tile_pool(name="bn1", bufs=6))
                w1res = load_w(w1p, w1_d, "w1s")
                for pair in range(NST // 2):
                    sbs = []
                    for ob in range(NB):
                        ps = conv(w1res, xpad, pair, ob)
                        sb = bounce.tile([P, 1024], BF16, tag="bn",
                                         name=f"sb{ob}")
                        for sl in range(2):
                            nc.scalar.activation(
                                sb[:, sl * 512:(sl + 1) * 512], ps[sl][:],
                                AF.Relu, bias=b1_t[ob][:])
                        # resident s1 with v-bias folded (residual term)
                        nc.scalar.activation(
                            s1r[ob][:, pair * 1024:(pair + 1) * 1024],
                            sb[:], AF.Identity, bias=vba_t[ob][:])
                        sbs.append(sb)
                    for sl in range(2):
                        st = pair * 2 + sl
                        ssl = slice(sl * 512, (sl + 1) * 512)
                        pq = psA.tile([CI, 512], F32, tag="r5", name="pq")
                        pk = psA.tile([CI, 512], F32, tag="r5", name="pk")
                        for cb in range(NB):
                            nc.tensor.matmul(pq[:], qw_t[cb][:],
                                             sbs[cb][:, ssl],
                                             start=(cb == 0),
                                             stop=(cb == NB - 1))
                        nc.scalar.activation(
                            qsb[:, st * 512:(st + 1) * 512], pq[:],
                            AF.Identity, bias=qb_t[:])
                        for cb in range(NB):
                            nc.tensor.matmul(pk[:], kw_t[cb][:],
                                             sbs[cb][:, ssl],
                                             start=(cb == 0),
                                             stop=(cb == NB - 1))
                        nc.scalar.activation(
                            kaug[0:CI, st * 512:(st + 1) * 512], pk[:],
                            AF.Identity, bias=kb_t[:])
                        for j in range(4):
                            ch = st * 4 + j
                            pv = psA.tile([P, 512], F32, tag="r5", name="pv")
                            for cb in range(NB):
                                nc.tensor.matmul(
                                    pv[:],
                                    sbs[cb][:, sl * 512 + j * P:
                                            sl * 512 + (j + 1) * P],
                                    vw_t[cb][:], start=(cb == 0),
                                    stop=(cb == NB - 1))
                            nc.scalar.activation(
                                vt[ch // 8][:, (ch % 8) * 512:
                                            (ch % 8 + 1) * 512],
                                pv[:], AF.Identity)

            # ---- attention (transposed-logits flow) + conv2 ----
            # one stack so w2res (loaded during attention) survives conv2
            with ExitStack() as at:
                w2p = at.enter_context(tc.tile_pool(name="w2p", bufs=NB))
                qgp = at.enter_context(tc.tile_pool(name="qgp", bufs=2))
                prb = at.enter_context(tc.tile_pool(name="prb", bufs=3))
                stp = at.enter_context(tc.tile_pool(name="stp", bufs=4))
                tmp = at.enter_context(tc.tile_pool(name="tmp", bufs=3))
                w2res = load_w(w2p, w2_d, "w2s")
                for g in range(NST):
                    gsl = slice(g * 512, (g + 1) * 512)
                    # pass A: per-column (kq) max via normal-layout logits
                    nm4 = stp.tile([P, NB], BF16, tag="nm", name="nm4")
                    for blk in range(NB):
                        pmax = stp.tile([P, NST], F32, tag="pm", name="pmax")
                        for st in range(NST):
                            pl = psA.tile([P, 512], F32, tag="r5", name="pl")
                            nc.tensor.matmul(
                                pl[:],
                                qsb[:, g * 512 + blk * P:
                                    g * 512 + (blk + 1) * P],
                                kaug[0:CI, st * 512:(st + 1) * 512],
                                start=True, stop=True)
                            nc.vector.reduce_max(pmax[:, st:st + 1], pl[:],
                                                 axis=AX.X)
                        nc.vector.reduce_max(nm4[:, blk:blk + 1], pmax[:],
                                             axis=AX.X, negate=True)
                    qaug = qgp.tile([CI + 1, 512], BF16, tag="qa",
                                    name="qaug")
                    nc.scalar.activation(qaug[0:CI, :], qsb[:, gsl], AF.Copy)
                    for blk in range(NB):
                        # [128,1] -> [1,128] so the PSUM read starts at
                        # partition 0 (engine partition-offset constraint)
                        ptm = psT.tile([1, P], BF16, tag="tp", name="ptm")
                        nc.tensor.transpose(ptm[:], nm4[:, blk:blk + 1],
                                            idb_t[:])
                        nc.scalar.activation(
                            qaug[CI:CI + 1, blk * P:(blk + 1) * P],
                            ptm[:], AF.Copy)
                    # pass B: transposed logits -> exp -> o accumulation
                    po = [psO.tile([P, 512], F32, tag="o5", name=f"po{i}")
                          for i in range(NB)]
                    psS = psN.tile([1, 512], F32, tag="nb", name="psS")
                    for j in range(NCH):
                        psL = psA.tile([P, 512], F32, tag="r5", name="psL")
                        nc.tensor.matmul(psL[:],
                                         kaug[:, j * P:(j + 1) * P],
                                         qaug[:], start=True, stop=True)
                        pb = prb.tile([P, 512], BF16, tag="pb", name="pb")
                        nc.scalar.activation(pb[:], psL[:], AF.Exp)
                        for cb in range(NB):
                            nc.tensor.matmul(
                                po[cb][:],
                                vt[j // 8][:, (j % 8) * 512 + cb * P:
                                           (j % 8) * 512 + (cb + 1) * P],
                                pb[:], start=(j == 0), stop=(j == NCH - 1))
                        nc.tensor.matmul(psS[:], onesc_t[:], pb[:],
                                         start=(j == 0), stop=(j == NCH - 1))
                    recip = stp.tile([1, 512], F32R, tag="rc", name="recip")
                    # f32r is bit-identical to f32; tag only affects PE rate
                    with nc.allow_low_precision(reason="f32r == f32 bits"):
                        nc.vector.reciprocal(recip[:], psS[:])
                    psB = psN.tile([P, 512], F32, tag="nb", name="psB")
                    nc.tensor.matmul(psB[:], onesr_t[:], recip[:],
                                     start=True, stop=True)
                    bc = tmp.tile([P, 512], BF16, tag="bc", name="bcsb")
                    nc.scalar.activation(bc[:], psB[:], AF.Copy)
                    for cb in range(NB):
                        t1 = tmp.tile([P, 512], BF16, tag="t1", name="t1")
                        nc.vector.tensor_mul(t1[:], po[cb][:], bc[:])
                        nc.vector.tensor_add(
                            _pv(xpad[cb][:], g * 8 + 1, 1, 8),
                            t1[:], s1r[cb][:, gsl])

                # ---- conv2 (same stack: w2res alive) ----
                conv2_phase(xpad, w2res, at)
            pctx.close()

        def channel_body():
            pctx = ExitStack()
            xpadp = pctx.enter_context(tc.tile_pool(name="xpadc", bufs=NB))
            s1p = pctx.enter_context(tc.tile_pool(name="s1pc", bufs=NB))
            xpad = load_xpad(xpadp)
            s1r = [s1p.tile([P, S], BF16, tag="s1", name=f"c1r{i}")
                   for i in range(NB)]

            with ExitStack() as c1:
                w1p = c1.enter_context(tc.tile_pool(name="w1pc", bufs=NB))
                w1res = load_w(w1p, w1_d, "w1c")
                for pair in range(NST // 2):
                    for ob in range(NB):
                        ps = conv(w1res, xpad, pair, ob)
                        for sl in range(2):
                            nc.scalar.activation(
                                s1r[ob][:, (pair * 2 + sl) * 512:
                                        (pair * 2 + sl + 1) * 512],
                                ps[sl][:], AF.Relu, bias=b1_t[ob][:])

            with ExitStack() as md:
                w2p = md.enter_context(tc.tile_pool(name="w2pc", bufs=NB))
                ctp = md.enter_context(tc.tile_pool(name="ctp", bufs=2))
                cap = md.enter_context(tc.tile_pool(name="cap", bufs=2 * NB))
                stp = md.enter_context(tc.tile_pool(name="stpc", bufs=8))
                w2res = load_w(w2p, w2_d, "w2c")
                # Gram via on-the-fly transposes of s1 chunks
                pg = [psO.tile([P, 512], F32, tag="o5", name=f"pg{i}")
                      for i in range(NB)]
                for j in range(NCH):
                    ct = ctp.tile([P, 512], BF16, tag="ct", name="ctj")
                    for cb in range(NB):
                        pt = psT.tile([P, P], BF16, tag="tp", name="ptc")
                        nc.tensor.transpose(
                            pt[:], s1r[cb][:, j * P:(j + 1) * P], idb_t[:])
                        nc.scalar.activation(ct[:, cb * P:(cb + 1) * P],
                                             pt[:], AF.Copy)
                    for cb in range(NB):
                        nc.tensor.matmul(pg[cb][:], ct[:, cb * P:(cb + 1) * P],
                                         ct[:], start=(j == 0),
                                         stop=(j == NCH - 1))
                # row-softmax with beta folded
                catt = []
                for cb in range(NB):
                    negmax = stp.tile([P, 1], F32, tag="st", name="negmax")
                    nc.vector.reduce_max(negmax[:], pg[cb][:], axis=AX.X,
                                         negate=True)
                    ctile = cap.tile([P, 512], BF16, tag="ca",
                                     name=f"catt{cb}")
                    rowsum = stp.tile([P, 1], F32, tag="st", name="rowsum")
                    nc.scalar.activation(ctile[:], pg[cb][:], AF.Exp,
                                         bias=negmax[:], accum_out=rowsum[:])
                    recip = stp.tile([P, 1], F32, tag="st", name="recip")
                    nc.vector.reciprocal(recip[:], rowsum[:])
                    nc.vector.tensor_mul(recip[:], recip[:], beta_t[:])
                    nc.scalar.activation(ctile[:], ctile[:], AF.Identity,
                                         scale=recip[:])
                    catt.append(ctile)
                # transpose catt -> cattT (correct M @ F orientation)
                cattT = [cap.tile([P, 512], BF16, tag="caT", name=f"caT{i}")
                         for i in range(NB)]
                for kb in range(NB):
                    for cb in range(NB):
                        pt = psT.tile([P, P], BF16, tag="tp", name="ptc2")
                        nc.tensor.transpose(
                            pt[:], catt[kb][:, cb * P:(cb + 1) * P], idb_t[:])
                        nc.scalar.activation(
                            cattT[cb][:, kb * P:(kb + 1) * P], pt[:], AF.Copy)
                # apply: co = M @ F, residual into xpad
                for st in range(NST):
                    ssl = slice(st * 512, (st + 1) * 512)
                    for kb in range(NB):
                        pc = psA.tile([P, 512], F32, tag="r5", name="pc")
                        for cb in range(NB):
                            nc.tensor.matmul(
                                pc[:], cattT[cb][:, kb * P:(kb + 1) * P],
                                s1r[cb][:, ssl], start=(cb == 0),
                                stop=(cb == NB - 1))
                        nc.vector.tensor_add(
                            _pv(xpad[kb][:], st * 8 + 1, 1, 8),
                            pc[:], s1r[kb][:, ssl])

                conv2_phase(xpad, w2res, md)
            pctx.close()

        def body():
            if branch == "spatial":
                spatial_body()
            elif branch == "channel":
                channel_body()
            else:
                pid = nc.partition_id()
                with tc.If(pid < 4) as cmp:
                    spatial_body()
                with cmp.Else():
                    channel_body()

        if reps == 1:
            body()
        else:
            with tc.For_i(0, reps):
                body()

        gctx.close()

    nc.compile()
    return nc


def _fold_conv(w, g, b, m, v):
    import ml_dtypes
    scale = np.asarray(g) / np.sqrt(np.asarray(v) + EPS)
    wf = (np.asarray(w) * scale[:, None, None, None]).astype(np.float32)
    bf = (np.asarray(b) - np.asarray(m) * scale).astype(np.float32)
    # [O, Ci, 3, 3] -> per-cb resident layout [cb][ci, ob*1152 + tap*128 + o]
    wfo = wf.reshape(NB, P, NB, P, 3, 3)
    wl = wfo.transpose(2, 3, 0, 4, 5, 1).reshape(NB, P, NB * 9 * P)
    return np.ascontiguousarray(wl.astype(ml_dtypes.bfloat16)), \
        bf.reshape(NB, P, 1)


def _pad_x(x):
    import ml_dtypes
    xp = np.zeros((NB, P, PR, PW), ml_dtypes.bfloat16)
    xp[:, :, 1:65, 1:65] = np.asarray(x, np.float32).reshape(NB, P, H, W)
    return xp.reshape(NB, P, PAD)


def prep_inputs(inputs):
    import ml_dtypes
    BD = ml_dtypes.bfloat16
    x = np.asarray(inputs["x"], np.float32)
    alpha = float(np.asarray(inputs["alpha"]).reshape(-1)[0])
    beta = float(np.asarray(inputs["beta"]).reshape(-1)[0])

    w1s, b1s = _fold_conv(inputs["sa_w1"], inputs["sa_g1"], inputs["sa_b1"],
                          inputs["sa_m1"], inputs["sa_v1"])
    w2s, b2s = _fold_conv(inputs["sa_w2"], inputs["sa_g2"], inputs["sa_b2"],
                          inputs["sa_m2"], inputs["sa_v2"])
    w1c, b1c = _fold_conv(inputs["ca_w1"], inputs["ca_g1"], inputs["ca_b1"],
                          inputs["ca_m1"], inputs["ca_v1"])
    w2c, b2c = _fold_conv(inputs["ca_w2"], inputs["ca_g2"], inputs["ca_b2"],
                          inputs["ca_m2"], inputs["ca_v2"])

    qw = np.ascontiguousarray(
        np.asarray(inputs["q_w"], np.float32).T.reshape(NB, P, CI)).astype(BD)
    kw = np.ascontiguousarray(
        np.asarray(inputs["k_w"], np.float32).T.reshape(NB, P, CI)).astype(BD)
    vw = np.ascontiguousarray(
        (alpha * np.asarray(inputs["v_w"], np.float32)).T.reshape(
            NB, P, 512)).astype(BD)
    qb = np.asarray(inputs["q_b"], np.float32).reshape(CI, 1)
    kb = np.asarray(inputs["k_b"], np.float32).reshape(CI, 1)
    vba = (alpha * np.asarray(inputs["v_b"], np.float32)).reshape(NB, P, 1)
    betat = np.full((P, 1), beta, np.float32)
    identb = np.eye(P, dtype=BD)
    onesc = np.ones((P, 1), BD)
    onesr = np.ones((1, P), np.float32)
    onesk = np.ones((1, S), BD)

    zqw = np.zeros_like(qw)
    zvw = np.zeros_like(vw)
    zb = np.zeros_like(qb)
    zvba = np.zeros_like(vba)

    maps = []
    for core in range(8):
        b = core % 4
        xp = _pad_x(x[b])
        common = dict(xpad=xp, betat=betat, identb=identb, onesc=onesc,
                      onesr=onesr, onesk=onesk)
        if core < 4:
            m = dict(w1=w1s, b1=b1s, w2=w2s, b2=b2s, qw=qw, kw=kw, vw=vw,
                     qb=qb, kb=kb, vba=vba, **common)
        else:
            m = dict(w1=w1c, b1=b1c, w2=w2c, b2=b2c, qw=zqw, kw=zqw, vw=zvw,
                     qb=zb, kb=zb, vba=zvba, **common)
        maps.append(m)
    return maps


def kernel(**inputs):
    if "nc" not in _CACHE:
        _CACHE["nc"] = build()
    nc = _CACHE["nc"]
    maps = prep_inputs(inputs)
    # first execution of a fresh NEFF occasionally wedges the device
    # (NRT_EXEC_UNIT_UNRECOVERABLE); the wedge is transient, so retry
    # with a short backoff before giving up
    import time
    last = None
    for attempt in range(4):
        try:
            res = run_bass_kernel_spmd(nc, maps, core_ids=list(range(8)))
            break
        except Exception as e:
            last = e
            time.sleep(5 * (attempt + 1))
    else:
        raise last
    out = np.zeros((B, C, H, W), np.float32)
    for b in range(B):
        sa = res.results[b]["out"].reshape(C, H, W)
        ca = res.results[b + 4]["out"].reshape(C, H, W)
        out[b] = sa + ca
    return out
